# revision 21
# baseline (speedup 1.0000x reference)
"""Trainium2 Bass kernel for the guided-diffusion AttentionBlock (fp8 version).

Shapes: x (8, 512, 32, 32) fp32, GroupNorm(32), 8 heads (head dim 64),
qkv 1x1 conv (1536x512), proj 1x1 conv (512x512), residual add.
Sharding: pure data-parallel, one batch item per NeuronCore, no collectives.

Key design points (vs the fp16 baseline this replaces):
  - All heavy matmuls run fp8e4m3 with perf_mode=DoubleRow: operands carry
    two free-dim "slabs" contracted over the same partitions (virtual
    K = 2*128), so each PE cell does 2 MACs/cycle.  The chip's sustained
    8-core power limiter caps PE activity at ~8192 cell-cols/cycle, so
    MACs-per-cell-cycle is the only throughput lever that survives the
    throttle; DoubleRow halves the dense-phase cell-cycles.
  - Scores: q/k are folded (via a DRAM bounce) to a [32-partition, 2-slab,
    L] per-head layout so each score matmul is a DoubleRow K=64(virt)
    instruction on 32 PE rows; two heads run concurrently via
    tile_position row packing.
  - Softmax exp is split across ScalarE (true exp, fp8 out) and VectorE
    (Schraudolph-style exp: one tensor_scalar mult+add with int8 output
    whose bit pattern IS the fp8e4m3 encoding of exp(x)/2; ~4% rms error,
    harmless at the 2e-2 gate).  Both paths compute exp(x)*0.5 (softmax
    shift-invariant) to keep the int8 bits in [0, 126].
  - The softmax denominator rides along as a ones-column in vhat (row 64
    of the attention output PSUM).  1/denom via reciprocal_approx_fast
    on the denominator row, then a DRAM-bounce partition-broadcast, then
    one scalar_tensor_tensor per head does (aun * 16) * recip -> fp8 a.
  - Weights/activations carry power-of-2 scale factors chosen to center
    fp8 ranges (Sq=Sk=16, Sv=4, a boost 16, wp x8); descales fold into
    the exp scale and the final (o/512 + x) residual scalar_tensor_tensor.
  - proj bias via a K=1 rank-1 fp8 matmul into the proj PSUM group;
    q/k biases fold into the ACT PSUM evacuations ([p,1] bias).

Environment note: the TileContext epilogue's EVENT_SEMAPHORE_RANGE_CLEAR
crashes this runtime's exec unit, so clear_and_free_semaphores is patched
to per-semaphore sem-wr-imm writes on gpsimd NOPs.
"""

import math
import sys

if "/opt/trn_rl_repo" not in sys.path:
    sys.path.insert(0, "/opt/trn_rl_repo")

import numpy as np
import ml_dtypes

import concourse.bass as bass
import concourse.bacc as bacc
import concourse.mybir as mybir
import concourse.tile as tile
from concourse.bass_utils import run_bass_kernel_spmd

B, C, H, W = 8, 512, 32, 32
L = H * W               # 1024
N_HEADS = 8
CH = C // N_HEADS       # 64
N_GROUPS = 32
GSIZE = C // N_GROUPS   # 16
CB = C // 128           # 4 channel blocks
NG_BLK = 128 // GSIZE   # 8 groups per channel block
LT = L // 128           # 8 l-tiles
EPS = 1e-5

F32 = mybir.dt.float32
F16 = mybir.dt.float16
F8 = mybir.dt.float8e4
I8 = mybir.dt.int8
AX = mybir.AxisListType
AF = mybir.ActivationFunctionType
ALU = mybir.AluOpType
DR = mybir.MatmulPerfMode.DoubleRow

# fp8 scale plan
SATT = 1.0 / math.sqrt(math.sqrt(CH))   # attention 1/sqrt(sqrt(ch)), folded
SQ = 16.0                                # q carries SQ*SATT*q_true
SK = 16.0                                # k carries SK*SATT*k_true
SV = 4.0                                 # v carries SV*v_true
SAB = 16.0                               # post-division boost: a = SV*SAB*a_true
SPW = 8.0                                # wp host premultiplier
SPT = SV * SAB * SPW                     # proj PSUM carries SPT*o_true (=512)
SC_SCL = 1.0 / (SQ * SK)                 # score PSUM -> true score
LN2 = math.log(2.0)
EXP_BIAS = -LN2                          # ACT computes exp(score)-shifted = exp*0.5
KS_DVE = 8.0 / LN2 * SC_SCL              # int8 schraudolph multiplier
BS_DVE = 48.0                            # 56 - 8  (same *0.5 shift as ACT path)
VH = 80                                  # per-head stride in vhat (64 v + ones + pad)


def _patch_sem_clear():
    """Replace the RANGE_CLEAR epilogue with per-sem sem-wr-imm NOPs."""
    if getattr(bass.Bass, "_ant_semclear_patched", False):
        return

    def clear_and_free_semaphores(self, sems):
        if not sems:
            return
        sem_nums = [
            s.num if isinstance(s, bass.SemaphoreHandle) else s for s in sems
        ]
        for num in sem_nums:
            inst = self.gpsimd.nop(nofuse=True)
            si = inst.ins.sync_info
            if si is None:
                si = mybir.SyncInfo(on_wait=[], on_update=[])
                inst.ins.sync_info = si
            si.on_update.append(
                mybir.SyncUpdate(
                    sync_type="semaphore",
                    id=num,
                    update_mode="sem-wr-imm",
                    update_value=0,
                )
            )
        self._state.prepend_free_semaphores(sem_nums)
        for poison_set in self._tile_sem_poison_stack:
            poison_set.update(sem_nums)

    bass.Bass.clear_and_free_semaphores = clear_and_free_semaphores
    bass.Bass._ant_semclear_patched = True


def build_program():
    _patch_sem_clear()
    nc = bacc.Bacc("TRN2", target_bir_lowering=False, debug=False)

    x_d = nc.declare_dram_parameter("x", [C, L], F32, isOutput=False)
    # DR weight tiles: [kp][128, 2, 512] with slab j = input-channel block 2kp+j
    w_d = {}
    for nm in ("wq", "wk", "wv", "wp"):
        for kp in range(2):
            w_d[(nm, kp)] = nc.declare_dram_parameter(
                f"{nm}{kp}", [128, 2 * C], F8, isOutput=False)
    bq_d = nc.declare_dram_parameter("bq", [1, C], F32, isOutput=False)
    bk_d = nc.declare_dram_parameter("bk", [1, C], F32, isOutput=False)
    bv_d = nc.declare_dram_parameter("bv", [1, C], F8, isOutput=False)
    bp_d = nc.declare_dram_parameter("bp", [1, C], F32, isOutput=False)
    gam_d = nc.declare_dram_parameter("gamma", [CB, 128], F32, isOutput=False)
    bet_d = nc.declare_dram_parameter("beta", [CB, 128], F32, isOutput=False)
    out_d = nc.declare_dram_parameter("out", [C, L], F32, isOutput=True)

    # one-hot group selector (channel-in-block -> group-in-block) and its T
    g_np = np.zeros((128, NG_BLK), dtype=np.float32)
    for c in range(128):
        g_np[c, c // GSIZE] = 1.0
    g_d = nc.inline_tensor(g_np, name="gsel")
    gt_d = nc.inline_tensor(np.ascontiguousarray(g_np.T), name="gselT")
    # DRAM scratch: q/k staging for the 32-partition score fold, recip rows
    recip_d = nc.dram_tensor("recip_scratch", [N_HEADS, L], F32)

    exp_ctr = [0]

    def exp_engine():
        exp_ctr[0] += 1
        return "dve" if exp_ctr[0] % 2 == 0 else "act"

    with tile.TileContext(nc) as tc:
        with (
            tc.tile_pool(name="per", bufs=1) as per,      # persistent sbuf
            tc.tile_pool(name="tmp", bufs=2) as tmp,      # transient sbuf
        ):
            # ---------- loads ----------
            x_sb = [per.tile([128, L], F32, name=f"x{i}") for i in range(CB)]
            for cb in range(CB):
                eng = nc.sync if cb % 2 == 0 else nc.scalar
                eng.dma_start(out=x_sb[cb], in_=x_d.ap()[cb * 128:(cb + 1) * 128, :])

            w_sb = {}
            for nm, kp in w_d:
                w_sb[(nm, kp)] = per.tile([128, 2, C], F8, name=f"{nm}{kp}")
                eng = nc.sync if nm in ("wq", "wk") else nc.scalar
                eng.dma_start(out=w_sb[(nm, kp)], in_=w_d[(nm, kp)].ap())

            brow = {}
            for nm, d in (("bv", bv_d),):
                brow[nm] = per.tile([1, C], F8, name=f"{nm}r")
                nc.scalar.dma_start(out=brow[nm], in_=d.ap())
            bp_col = per.tile([128, CB], F32, name="bp_col")
            for ob in range(CB):
                nc.scalar.dma_start(out=bp_col[:, ob:ob + 1],
                                    in_=bp_d.ap()[0, ob * 128:(ob + 1) * 128])

            bq_col = per.tile([128, CB], F32, name="bq_col")
            bk_col = per.tile([128, CB], F32, name="bk_col")
            for ob in range(CB):
                nc.scalar.dma_start(out=bq_col[:, ob:ob + 1],
                                    in_=bq_d.ap()[0, ob * 128:(ob + 1) * 128])
                nc.scalar.dma_start(out=bk_col[:, ob:ob + 1],
                                    in_=bk_d.ap()[0, ob * 128:(ob + 1) * 128])
            gam_sb = per.tile([128, CB], F32, name="gam")
            bet_sb = per.tile([128, CB], F32, name="bet")
            for cb in range(CB):
                nc.scalar.dma_start(out=gam_sb[:, cb:cb + 1], in_=gam_d.ap()[cb])
                nc.scalar.dma_start(out=bet_sb[:, cb:cb + 1], in_=bet_d.ap()[cb])

            g_sb = per.tile([128, NG_BLK], F32, name="gsel")
            nc.scalar.dma_start(out=g_sb, in_=g_d.ap())
            gt_sb = per.tile([NG_BLK, 128], F32, name="gselT")
            nc.scalar.dma_start(out=gt_sb, in_=gt_d.ap())

            ones_row = per.tile([1, C], F8, name="ones_row")
            nc.vector.memset(ones_row, 1.0)
            eps_sb = per.tile([NG_BLK, 1], F32, name="eps")
            nc.vector.memset(eps_sb, EPS)
            expb_col = per.tile([128, 1], F32, name="expb")
            nc.vector.memset(expb_col, EXP_BIAS)

            # vhat: per s-block-pair [128, 2, 8*VH]; head h at cols VH*h..,
            # col 64 = ones (denominator trick), cols 65..VH-1 zero pad
            vhat = [per.tile([128, 2, N_HEADS * VH], F8, name=f"vh{i}")
                    for i in range(LT // 2)]
            for sp in range(LT // 2):
                v4 = vhat[sp].rearrange("p s (h c) -> p s h c", c=VH)
                nc.vector.memset(v4[:, :, :, CH:CH + 1], 1.0)
                nc.vector.memset(v4[:, :, :, CH + 1:VH], 0.0)

            # ---------- GroupNorm (fp32, as baseline) ----------
            stats = per.tile([128, 2 * CB], F32, name="stats")
            # xn in DR slab layout: [kp][128, 2, L], slab j = channel blk 2kp+j
            xn = [per.tile([128, 2, L], F8, name=f"xn{i}") for i in range(2)]
            with tc.tile_pool(name="ps_gn", bufs=1, space="PSUM") as ps_gn:
                for cb in range(CB):
                    nc.vector.tensor_reduce(
                        out=stats[:, 2 * cb:2 * cb + 1], in_=x_sb[cb],
                        axis=AX.X, op=ALU.add,
                    )
                    sq_scr = tmp.tile([128, L], F32, name="sq_scr", tag="sq_scr")
                    nc.vector.scalar_tensor_tensor(
                        out=sq_scr, in0=x_sb[cb], scalar=1.0, in1=x_sb[cb],
                        op0=ALU.mult, op1=ALU.mult,
                        accum_out=stats[:, 2 * cb + 1:2 * cb + 2],
                    )
                gstat_ps = ps_gn.tile([NG_BLK, 2 * CB], F32, name="gstat")
                nc.tensor.matmul(gstat_ps, g_sb, stats, start=True, stop=True)

                inv_n = 1.0 / (GSIZE * L)
                mu = tmp.tile([NG_BLK, CB], F32, name="mu", bufs=1)
                ex2 = tmp.tile([NG_BLK, CB], F32, name="ex2", bufs=1)
                nc.vector.tensor_scalar_mul(mu, gstat_ps[:, 0::2], inv_n)
                nc.vector.tensor_scalar_mul(ex2, gstat_ps[:, 1::2], inv_n)
                var = tmp.tile([NG_BLK, CB], F32, name="var", bufs=1)
                nc.vector.tensor_mul(out=var, in0=mu, in1=mu)
                nc.vector.tensor_sub(out=var, in0=ex2, in1=var)
                nc.scalar.activation(out=var, in_=var, func=AF.Sqrt, bias=eps_sb)
                rs = tmp.tile([NG_BLK, CB], F32, name="rs", bufs=1)
                nc.vector.reciprocal(out=rs, in_=var)
                rbc = tmp.tile([NG_BLK, 2 * CB], F32, name="rbc", bufs=1)
                nc.vector.tensor_copy(rbc[:, 0::2], rs)
                nc.vector.tensor_mul(out=rbc[:, 1::2], in0=mu, in1=rs)
                chan_ps = ps_gn.tile([128, 2 * CB], F32, name="chan")
                nc.tensor.matmul(chan_ps, gt_sb, rbc, start=True, stop=True)

                ab = per.tile([128, 2 * CB], F32, name="ab")
                nc.vector.tensor_mul(out=ab[:, 0::2], in0=chan_ps[:, 0::2], in1=gam_sb)
                nc.vector.tensor_mul(out=ab[:, 1::2], in0=chan_ps[:, 1::2], in1=gam_sb)
                nc.vector.tensor_sub(out=ab[:, 1::2], in0=bet_sb, in1=ab[:, 1::2])
                for cb in range(CB):
                    nc.scalar.activation(
                        out=xn[cb // 2][:, cb % 2, :], in_=x_sb[cb],
                        func=AF.Identity,
                        scale=ab[:, 2 * cb:2 * cb + 1],
                        bias=ab[:, 2 * cb + 1:2 * cb + 2],
                    )
                # preload the exp table set before the attention phase
                junk = tmp.tile([1, 8], F32, name="junk", bufs=1)
                nc.scalar.activation(out=junk, in_=stats[0:1, 0:8], func=AF.Exp,
                                     scale=0.0)

            # ---------- qkv ----------
            q_sb = [per.tile([128, L], F8, name=f"q{i}") for i in range(CB)]
            k_sb = [per.tile([128, L], F8, name=f"k{i}") for i in range(CB)]

            with tc.tile_pool(name="ps_qkv", bufs=1, space="PSUM") as ps_qkv:
                for nm, dst, bcol in (("wq", q_sb, bq_col),
                                      ("wk", k_sb, bk_col)):
                    for ob in range(CB):
                        for hf in range(2):
                            qk_ps = ps_qkv.tile([128, 512], F32, name="qk_ps",
                                                tag=f"qk{(ob * 2 + hf) % 3}", bufs=1)
                            for kp in range(2):
                                nc.tensor.matmul(
                                    qk_ps,
                                    w_sb[(nm, kp)][:, :, ob * 128:(ob + 1) * 128],
                                    xn[kp][:, :, hf * 512:(hf + 1) * 512],
                                    start=(kp == 0), stop=(kp == 1),
                                    perf_mode=DR,
                                )
                            if (ob + hf) % 2 == 0:
                                nc.scalar.activation(
                                    out=dst[ob][:, hf * 512:(hf + 1) * 512],
                                    in_=qk_ps, func=AF.Identity,
                                    bias=bcol[:, ob:ob + 1],
                                )
                            else:
                                nc.vector.tensor_scalar(
                                    out=dst[ob][:, hf * 512:(hf + 1) * 512],
                                    in0=qk_ps, scalar1=bcol[:, ob:ob + 1],
                                    scalar2=None, op0=ALU.add,
                                )

                for lt in range(LT):
                    v_ps = ps_qkv.tile([128, 512], F32, name="v_ps",
                                       tag=f"v{lt % 3}", bufs=1)
                    for kp in range(2):
                        nc.tensor.matmul(
                            v_ps,
                            xn[kp][:, :, lt * 128:(lt + 1) * 128],
                            w_sb[("wv", kp)],
                            start=(kp == 0), stop=False,
                            perf_mode=DR,
                        )
                    nc.tensor.matmul(
                        v_ps, ones_row[:, 0:128], brow["bv"],
                        start=False, stop=True,
                    )
                    vdst = vhat[lt // 2].rearrange(
                        "p s (h c) -> p s h c", c=VH)[:, lt % 2, :, 0:CH]
                    vsrc = v_ps.rearrange("p (h c) -> p h c", c=CH)
                    if lt % 2 == 0:
                        nc.scalar.activation(out=vdst, in_=vsrc,
                                             func=AF.Identity)
                    else:
                        nc.vector.tensor_copy(vdst, vsrc)

            # residual carries the proj bias: x += bp (emitted here so the
            # DVE work lands in the qkv phase, not the startup window)
            for cb in range(CB):
                nc.vector.tensor_scalar(
                    out=x_sb[cb], in0=x_sb[cb],
                    scalar1=bp_col[:, cb:cb + 1], scalar2=None,
                    op0=ALU.add)

            # ---------- attention ----------
            # a in DR slab layout for proj: [kp][128, 2, L]
            a_pair = [per.tile([128, 2, L], F8, name=f"ap{i}") for i in range(2)]
            with tc.tile_pool(name="ps_att", bufs=1, space="PSUM") as ps_att:
                sci = [0]
                for hp in range(N_HEADS // 2):
                    aun_sb = {}
                    for sub in range(2):
                        aun_sb[sub] = tmp.tile([VH, L], F32, name=f"aunsb{sub}",
                                               tag=f"aunsb{sub}", bufs=2)
                    for hf in range(2):
                        aun_ps = {}
                        for sub in range(2):
                            aun_ps[sub] = ps_att.tile([VH, 512], F32,
                                                      name=f"aun{sub}",
                                                      tag=f"aun{sub}", bufs=2)
                        for sp in range(LT // 2):
                            ex8 = {}
                            for sub in range(2):
                                ex8[sub] = tmp.tile([128, 2, 512], F8,
                                                    name="ex8",
                                                    tag=f"ex{sub}", bufs=2)
                            for stp in range(2):
                                st = 2 * sp + stp
                                for sub in range(2):
                                    pl = 64 * sub
                                    sc = ps_att.tile(
                                        [128, 512], F32, name="sc",
                                        tag=f"sc{sci[0] % 4}", bufs=1)
                                    sci[0] += 1
                                    nc.tensor.matmul(
                                        sc,
                                        k_sb[hp][pl:pl + 64,
                                                 st * 128:(st + 1) * 128],
                                        q_sb[hp][pl:pl + 64,
                                                 hf * 512:(hf + 1) * 512],
                                        start=True, stop=True,
                                        tile_position=(pl, 0),
                                    )
                                    if exp_engine() == "act":
                                        nc.scalar.activation(
                                            out=ex8[sub][:, stp, :], in_=sc,
                                            func=AF.Exp, scale=SC_SCL,
                                            bias=expb_col,
                                        )
                                    else:
                                        nc.vector.tensor_scalar(
                                            out=ex8[sub][:, stp, :].bitcast(I8),
                                            in0=sc, scalar1=KS_DVE,
                                            scalar2=BS_DVE,
                                            op0=ALU.mult, op1=ALU.add,
                                        )
                            for sub in range(2):
                                h = 2 * hp + sub
                                nc.tensor.matmul(
                                    aun_ps[sub][:, :],
                                    vhat[sp].rearrange(
                                        "p s (h c) -> p s h c", c=VH)[:, :, h, :],
                                    ex8[sub],
                                    start=(sp == 0), stop=(sp == LT // 2 - 1),
                                    perf_mode=DR,
                                )
                        for sub in range(2):
                            nc.scalar.activation(
                                out=aun_sb[sub][:, hf * 512:(hf + 1) * 512],
                                in_=aun_ps[sub], func=AF.Identity,
                            )

                    # denominator rows -> DRAM -> partition-broadcast back
                    # (engines cannot cross partitions, so the raw denominator
                    # is bounced through DRAM and the reciprocal happens on the
                    # broadcast [64, L] tile at base partition 0)
                    for sub in range(2):
                        h = 2 * hp + sub
                        nc.sync.dma_start(out=recip_d.ap()[h:h + 1, :],
                                            in_=aun_sb[sub][CH:CH + 1, :])
                    for sub in range(2):
                        h = 2 * hp + sub
                        kp, slab = h // 4, (h // 2) % 2
                        den_b = tmp.tile([CH, L], F32, name="den_b",
                                         tag=f"den{sub}", bufs=2)
                        bsrc = bass.AP(
                            tensor=recip_d.ap().tensor, offset=h * L,
                            ap=[[0, CH], [1, L]],
                        )
                        nc.sync.dma_start(out=den_b, in_=bsrc)
                        rec_b = tmp.tile([CH, L], F32, name="rec_b",
                                         tag=f"recb{sub}", bufs=2)
                        nc.vector.reciprocal_approx_fast(out=rec_b, in_=den_b)
                        if h % 2 == 0:
                            nc.vector.scalar_tensor_tensor(
                                out=a_pair[kp][0:CH, slab, :],
                                in0=aun_sb[sub][0:CH, :], scalar=SAB,
                                in1=rec_b, op0=ALU.mult, op1=ALU.mult,
                            )
                        else:
                            ahead = tmp.tile([CH, L], F8, name="ahead",
                                             tag="ahead", bufs=2)
                            nc.vector.scalar_tensor_tensor(
                                out=ahead, in0=aun_sb[sub][0:CH, :],
                                scalar=SAB, in1=rec_b,
                                op0=ALU.mult, op1=ALU.mult,
                            )
                            nc.sync.dma_start(
                                out=a_pair[kp][CH:128, slab, :], in_=ahead)

                # ---------- proj + residual (two waves; kp0 prefires
                # against a_pair[0] while the last pair's divisions run) ----
                for wave in range(2):
                    o_tiles = {}
                    for g in range(4):
                        ob, hf = (2 * wave + g // 2), g % 2
                        o_tiles[g] = ps_att.tile([128, 512], F32, name="o_ps",
                                                 tag=f"sc{g}", bufs=1)
                        nc.tensor.matmul(
                            o_tiles[g],
                            w_sb[("wp", 0)][:, :, ob * 128:(ob + 1) * 128],
                            a_pair[0][:, :, hf * 512:(hf + 1) * 512],
                            start=True, stop=False,
                            perf_mode=DR,
                        )
                    for g in range(4):
                        ob, hf = (2 * wave + g // 2), g % 2
                        nc.tensor.matmul(
                            o_tiles[g],
                            w_sb[("wp", 1)][:, :, ob * 128:(ob + 1) * 128],
                            a_pair[1][:, :, hf * 512:(hf + 1) * 512],
                            start=False, stop=True,
                            perf_mode=DR,
                        )
                        res = tmp.tile([128, 512], F32, name="res",
                                       tag="res", bufs=3)
                        nc.vector.scalar_tensor_tensor(
                            out=res, in0=o_tiles[g], scalar=1.0 / SPT,
                            in1=x_sb[ob][:, hf * 512:(hf + 1) * 512],
                            op0=ALU.mult, op1=ALU.add,
                        )
                        nc.sync.dma_start(
                            out=out_d.ap()[ob * 128:(ob + 1) * 128,
                                           hf * 512:(hf + 1) * 512],
                            in_=res,
                        )

    nc.compile()
    return nc


def _f8(x):
    return np.ascontiguousarray(x.astype(ml_dtypes.float8_e4m3fn))


def make_in_maps(x, gn_scale, gn_bias, qkv_w, qkv_b, proj_w, proj_b):
    xf = np.ascontiguousarray(np.asarray(x, dtype=np.float32).reshape(B, C, L))
    qkv_w = np.asarray(qkv_w, dtype=np.float32)
    qkv_b = np.asarray(qkv_b, dtype=np.float32)
    proj_w = np.asarray(proj_w, dtype=np.float32)
    proj_b = np.asarray(proj_b, dtype=np.float32)

    def dr_tiles(wT, scl):
        # wT: [c_in, c_out] -> per kp [128, 2*C] fp8 with slab j = blk 2kp+j
        out = {}
        for kp in range(2):
            t = np.stack([wT[256 * kp + 128 * j:256 * kp + 128 * (j + 1)]
                          for j in range(2)], axis=1) * scl
            out[kp] = _f8(t.reshape(128, 2 * C))
        return out

    wq = dr_tiles(qkv_w[0:C].T, SATT * SQ)
    wk = dr_tiles(qkv_w[C:2 * C].T, SATT * SK)
    wv = dr_tiles(qkv_w[2 * C:3 * C].T, SV)
    wp = dr_tiles(proj_w.T, SPW)

    common = {
        "bq": np.ascontiguousarray((qkv_b[0:C] * SATT * SQ).reshape(1, C)),
        "bk": np.ascontiguousarray((qkv_b[C:2 * C] * SATT * SK).reshape(1, C)),
        "bv": _f8((qkv_b[2 * C:3 * C] * SV).reshape(1, C)),
        "bp": np.ascontiguousarray(proj_b.reshape(1, C)),
        "gamma": np.ascontiguousarray(
            np.asarray(gn_scale, dtype=np.float32).reshape(CB, 128)),
        "beta": np.ascontiguousarray(
            np.asarray(gn_bias, dtype=np.float32).reshape(CB, 128)),
    }
    for nm, w in (("wq", wq), ("wk", wk), ("wv", wv), ("wp", wp)):
        for kp in range(2):
            common[f"{nm}{kp}"] = w[kp]
    return [{"x": np.ascontiguousarray(xf[b]), **common} for b in range(B)]


def run(inputs, trace=False, trace_kwargs=None):
    nc = build_program()
    in_maps = make_in_maps(**inputs)
    res = run_bass_kernel_spmd(
        nc, in_maps, list(range(B)), trace=trace, **(trace_kwargs or {})
    )
    out = np.stack([res.results[b]["out"] for b in range(B)], axis=0)
    return out.reshape(B, C, H, W), res


def kernel(**inputs):
    out, _ = run(inputs)
    return out


# revision 22
# speedup vs baseline: 1.1563x; 1.1563x over previous
"""Trainium2 Bass kernel for the guided-diffusion AttentionBlock (fp8 version).

Shapes: x (8, 512, 32, 32) fp32, GroupNorm(32), 8 heads (head dim 64),
qkv 1x1 conv (1536x512), proj 1x1 conv (512x512), residual add.
Sharding: pure data-parallel, one batch item per NeuronCore, no collectives.

Key design points (vs the fp16 baseline this replaces):
  - All heavy matmuls run fp8e4m3 with perf_mode=DoubleRow: operands carry
    two free-dim "slabs" contracted over the same partitions (virtual
    K = 2*128), so each PE cell does 2 MACs/cycle.  The chip's sustained
    8-core power limiter caps PE activity at ~8192 cell-cols/cycle, so
    MACs-per-cell-cycle is the only throughput lever that survives the
    throttle; DoubleRow halves the dense-phase cell-cycles.
  - Scores: q/k are folded (via a DRAM bounce) to a [32-partition, 2-slab,
    L] per-head layout so each score matmul is a DoubleRow K=64(virt)
    instruction on 32 PE rows; two heads run concurrently via
    tile_position row packing.
  - Softmax exp is split across ScalarE (true exp, fp8 out) and VectorE
    (Schraudolph-style exp: one tensor_scalar mult+add with int8 output
    whose bit pattern IS the fp8e4m3 encoding of exp(x)/2; ~4% rms error,
    harmless at the 2e-2 gate).  Both paths compute exp(x)*0.5 (softmax
    shift-invariant) to keep the int8 bits in [0, 126].
  - The softmax denominator rides along as a ones-column in vhat (row 64
    of the attention output PSUM).  1/denom via reciprocal_approx_fast
    on the denominator row, then a DRAM-bounce partition-broadcast, then
    one scalar_tensor_tensor per head does (aun * 16) * recip -> fp8 a.
  - Weights/activations carry power-of-2 scale factors chosen to center
    fp8 ranges (Sq=Sk=16, Sv=4, a boost 16, wp x8); descales fold into
    the exp scale and the final (o/512 + x) residual scalar_tensor_tensor.
  - proj bias via a K=1 rank-1 fp8 matmul into the proj PSUM group;
    q/k biases fold into the ACT PSUM evacuations ([p,1] bias).

Environment note: the TileContext epilogue's EVENT_SEMAPHORE_RANGE_CLEAR
crashes this runtime's exec unit, so clear_and_free_semaphores is patched
to per-semaphore sem-wr-imm writes on gpsimd NOPs.
"""

import math
import sys

if "/opt/trn_rl_repo" not in sys.path:
    sys.path.insert(0, "/opt/trn_rl_repo")

import numpy as np
import ml_dtypes

import concourse.bass as bass
import concourse.bacc as bacc
import concourse.mybir as mybir
import concourse.tile as tile
from concourse.bass_utils import run_bass_kernel_spmd

B, C, H, W = 8, 512, 32, 32
L = H * W               # 1024
N_HEADS = 8
CH = C // N_HEADS       # 64
N_GROUPS = 32
GSIZE = C // N_GROUPS   # 16
CB = C // 128           # 4 channel blocks
NG_BLK = 128 // GSIZE   # 8 groups per channel block
LT = L // 128           # 8 l-tiles
EPS = 1e-5

F32 = mybir.dt.float32
F16 = mybir.dt.float16
F8 = mybir.dt.float8e4
I8 = mybir.dt.int8
AX = mybir.AxisListType
AF = mybir.ActivationFunctionType
ALU = mybir.AluOpType
DR = mybir.MatmulPerfMode.DoubleRow

# fp8 scale plan
SATT = 1.0 / math.sqrt(math.sqrt(CH))   # attention 1/sqrt(sqrt(ch)), folded
SQ = 16.0                                # q carries SQ*SATT*q_true
SK = 16.0                                # k carries SK*SATT*k_true
SV = 4.0                                 # v carries SV*v_true
SAB = 16.0                               # post-division boost: a = SV*SAB*a_true
SPW = 8.0                                # wp host premultiplier
SPT = SV * SAB * SPW                     # proj PSUM carries SPT*o_true (=512)
SC_SCL = 1.0 / (SQ * SK)                 # score PSUM -> true score
LN2 = math.log(2.0)
EXP_BIAS = -LN2                          # ACT computes exp(score)-shifted = exp*0.5
KS_DVE = 8.0 / LN2 * SC_SCL              # int8 schraudolph multiplier
BS_DVE = 48.0                            # 56 - 8  (same *0.5 shift as ACT path)
VH = 80                                  # per-head stride in vhat (64 v + ones + pad)


def _patch_sem_clear():
    """Replace the RANGE_CLEAR epilogue with per-sem sem-wr-imm NOPs."""
    if getattr(bass.Bass, "_ant_semclear_patched", False):
        return

    def clear_and_free_semaphores(self, sems):
        if not sems:
            return
        sem_nums = [
            s.num if isinstance(s, bass.SemaphoreHandle) else s for s in sems
        ]
        for num in sem_nums:
            inst = self.gpsimd.nop(nofuse=True)
            si = inst.ins.sync_info
            if si is None:
                si = mybir.SyncInfo(on_wait=[], on_update=[])
                inst.ins.sync_info = si
            si.on_update.append(
                mybir.SyncUpdate(
                    sync_type="semaphore",
                    id=num,
                    update_mode="sem-wr-imm",
                    update_value=0,
                )
            )
        self._state.prepend_free_semaphores(sem_nums)
        for poison_set in self._tile_sem_poison_stack:
            poison_set.update(sem_nums)

    bass.Bass.clear_and_free_semaphores = clear_and_free_semaphores
    bass.Bass._ant_semclear_patched = True


def build_program():
    _patch_sem_clear()
    nc = bacc.Bacc("TRN2", target_bir_lowering=False, debug=False)

    x_d = nc.declare_dram_parameter("x", [C, L], F32, isOutput=False)
    # DR weight tiles: [kp][128, 2, 512] with slab j = input-channel block 2kp+j
    w_d = {}
    for nm in ("wq", "wk", "wv", "wp"):
        for kp in range(2):
            w_d[(nm, kp)] = nc.declare_dram_parameter(
                f"{nm}{kp}", [128, 2 * C], F8, isOutput=False)
    bq_d = nc.declare_dram_parameter("bq", [1, C], F32, isOutput=False)
    bk_d = nc.declare_dram_parameter("bk", [1, C], F32, isOutput=False)
    bv_d = nc.declare_dram_parameter("bv", [1, C], F8, isOutput=False)
    bp_d = nc.declare_dram_parameter("bp", [1, C], F32, isOutput=False)
    gam_d = nc.declare_dram_parameter("gamma", [CB, 128], F32, isOutput=False)
    bet_d = nc.declare_dram_parameter("beta", [CB, 128], F32, isOutput=False)
    out_d = nc.declare_dram_parameter("out", [C, L], F32, isOutput=True)

    # one-hot group selector (channel-in-block -> group-in-block) and its T
    g_np = np.zeros((128, NG_BLK), dtype=np.float32)
    for c in range(128):
        g_np[c, c // GSIZE] = 1.0
    g_d = nc.inline_tensor(g_np, name="gsel")
    gt_d = nc.inline_tensor(np.ascontiguousarray(g_np.T), name="gselT")
    # DRAM scratch: q/k staging for the 32-partition score fold, recip rows
    recip_d = nc.dram_tensor("recip_scratch", [N_HEADS, L], F32)

    exp_ctr = [0]

    def exp_engine():
        exp_ctr[0] += 1
        return "dve" if exp_ctr[0] % 2 == 0 else "act"

    with tile.TileContext(nc) as tc:
        with (
            tc.tile_pool(name="per", bufs=1) as per,      # persistent sbuf
            tc.tile_pool(name="tmp", bufs=2) as tmp,      # transient sbuf
        ):
            # ---------- loads ----------
            x_sb = [per.tile([128, L], F32, name=f"x{i}") for i in range(CB)]
            for cb in range(CB):
                eng = nc.sync if cb % 2 == 0 else nc.scalar
                eng.dma_start(out=x_sb[cb], in_=x_d.ap()[cb * 128:(cb + 1) * 128, :])

            w_sb = {}
            for nm, kp in w_d:
                w_sb[(nm, kp)] = per.tile([128, 2, C], F8, name=f"{nm}{kp}")
                eng = nc.sync if nm in ("wq", "wk") else nc.scalar
                eng.dma_start(out=w_sb[(nm, kp)], in_=w_d[(nm, kp)].ap())

            brow = {}
            for nm, d in (("bv", bv_d),):
                brow[nm] = per.tile([1, C], F8, name=f"{nm}r")
                nc.scalar.dma_start(out=brow[nm], in_=d.ap())
            bp_col = per.tile([128, CB], F32, name="bp_col")
            for ob in range(CB):
                nc.scalar.dma_start(out=bp_col[:, ob:ob + 1],
                                    in_=bp_d.ap()[0, ob * 128:(ob + 1) * 128])

            bq_col = per.tile([128, CB], F32, name="bq_col")
            bk_col = per.tile([128, CB], F32, name="bk_col")
            for ob in range(CB):
                nc.scalar.dma_start(out=bq_col[:, ob:ob + 1],
                                    in_=bq_d.ap()[0, ob * 128:(ob + 1) * 128])
                nc.scalar.dma_start(out=bk_col[:, ob:ob + 1],
                                    in_=bk_d.ap()[0, ob * 128:(ob + 1) * 128])
            gam_sb = per.tile([128, CB], F32, name="gam")
            bet_sb = per.tile([128, CB], F32, name="bet")
            for cb in range(CB):
                nc.scalar.dma_start(out=gam_sb[:, cb:cb + 1], in_=gam_d.ap()[cb])
                nc.scalar.dma_start(out=bet_sb[:, cb:cb + 1], in_=bet_d.ap()[cb])

            g_sb = per.tile([128, NG_BLK], F32, name="gsel")
            nc.scalar.dma_start(out=g_sb, in_=g_d.ap())
            gt_sb = per.tile([NG_BLK, 128], F32, name="gselT")
            nc.scalar.dma_start(out=gt_sb, in_=gt_d.ap())

            ones_row = per.tile([1, C], F8, name="ones_row")
            nc.vector.memset(ones_row, 1.0)
            eps_sb = per.tile([NG_BLK, 1], F32, name="eps")
            nc.vector.memset(eps_sb, EPS)
            expb_col = per.tile([128, 1], F32, name="expb")
            nc.vector.memset(expb_col, EXP_BIAS)

            # vhat: per s-block-pair [128, 2, 8*VH]; head h at cols VH*h..,
            # col 64 = ones (denominator trick), cols 65..VH-1 zero pad
            vhat = [per.tile([128, 2, N_HEADS * VH], F8, name=f"vh{i}")
                    for i in range(LT // 2)]
            for sp in range(LT // 2):
                v4 = vhat[sp].rearrange("p s (h c) -> p s h c", c=VH)
                nc.vector.memset(v4[:, :, :, CH:CH + 1], 1.0)
                nc.vector.memset(v4[:, :, :, CH + 1:VH], 0.0)

            # ---------- GroupNorm (fp32, as baseline) ----------
            stats = per.tile([128, 2 * CB], F32, name="stats")
            # xn in DR slab layout: [kp][128, 2, L], slab j = channel blk 2kp+j
            xn = [per.tile([128, 2, L], F8, name=f"xn{i}") for i in range(2)]
            with tc.tile_pool(name="ps_gn", bufs=1, space="PSUM") as ps_gn:
                for cb in range(CB):
                    nc.vector.tensor_reduce(
                        out=stats[:, 2 * cb:2 * cb + 1], in_=x_sb[cb],
                        axis=AX.X, op=ALU.add,
                    )
                    sq_scr = tmp.tile([128, L], F32, name="sq_scr", tag="sq_scr")
                    nc.vector.scalar_tensor_tensor(
                        out=sq_scr, in0=x_sb[cb], scalar=1.0, in1=x_sb[cb],
                        op0=ALU.mult, op1=ALU.mult,
                        accum_out=stats[:, 2 * cb + 1:2 * cb + 2],
                    )
                gstat_ps = ps_gn.tile([NG_BLK, 2 * CB], F32, name="gstat")
                nc.tensor.matmul(gstat_ps, g_sb, stats, start=True, stop=True)

                inv_n = 1.0 / (GSIZE * L)
                mu = tmp.tile([NG_BLK, CB], F32, name="mu", bufs=1)
                ex2 = tmp.tile([NG_BLK, CB], F32, name="ex2", bufs=1)
                nc.vector.tensor_scalar_mul(mu, gstat_ps[:, 0::2], inv_n)
                nc.vector.tensor_scalar_mul(ex2, gstat_ps[:, 1::2], inv_n)
                var = tmp.tile([NG_BLK, CB], F32, name="var", bufs=1)
                nc.vector.tensor_mul(out=var, in0=mu, in1=mu)
                nc.vector.tensor_sub(out=var, in0=ex2, in1=var)
                nc.scalar.activation(out=var, in_=var, func=AF.Sqrt, bias=eps_sb)
                rs = tmp.tile([NG_BLK, CB], F32, name="rs", bufs=1)
                nc.vector.reciprocal(out=rs, in_=var)
                rbc = tmp.tile([NG_BLK, 2 * CB], F32, name="rbc", bufs=1)
                nc.vector.tensor_copy(rbc[:, 0::2], rs)
                nc.vector.tensor_mul(out=rbc[:, 1::2], in0=mu, in1=rs)
                chan_ps = ps_gn.tile([128, 2 * CB], F32, name="chan")
                nc.tensor.matmul(chan_ps, gt_sb, rbc, start=True, stop=True)

                ab = per.tile([128, 2 * CB], F32, name="ab")
                nc.vector.tensor_mul(out=ab[:, 0::2], in0=chan_ps[:, 0::2], in1=gam_sb)
                nc.vector.tensor_mul(out=ab[:, 1::2], in0=chan_ps[:, 1::2], in1=gam_sb)
                nc.vector.tensor_sub(out=ab[:, 1::2], in0=bet_sb, in1=ab[:, 1::2])
                for cb in range(CB):
                    nc.scalar.activation(
                        out=xn[cb // 2][:, cb % 2, :], in_=x_sb[cb],
                        func=AF.Identity,
                        scale=ab[:, 2 * cb:2 * cb + 1],
                        bias=ab[:, 2 * cb + 1:2 * cb + 2],
                    )
                # preload the exp table set before the attention phase
                junk = tmp.tile([1, 8], F32, name="junk", bufs=1)
                nc.scalar.activation(out=junk, in_=stats[0:1, 0:8], func=AF.Exp,
                                     scale=0.0)

            # ---------- qkv ----------
            q_sb = [per.tile([128, L], F8, name=f"q{i}") for i in range(CB)]
            k_sb = [per.tile([128, L], F8, name=f"k{i}") for i in range(CB)]

            with tc.tile_pool(name="ps_qkv", bufs=1, space="PSUM") as ps_qkv:
                for nm, dst, bcol in (("wq", q_sb, bq_col),
                                      ("wk", k_sb, bk_col)):
                    for ob in range(CB):
                        for hf in range(2):
                            qk_ps = ps_qkv.tile([128, 512], F32, name="qk_ps",
                                                tag=f"qk{(ob * 2 + hf) % 3}", bufs=1)
                            for kp in range(2):
                                nc.tensor.matmul(
                                    qk_ps,
                                    w_sb[(nm, kp)][:, :, ob * 128:(ob + 1) * 128],
                                    xn[kp][:, :, hf * 512:(hf + 1) * 512],
                                    start=(kp == 0), stop=(kp == 1),
                                    perf_mode=DR,
                                )
                            nc.scalar.activation(
                                out=dst[ob][:, hf * 512:(hf + 1) * 512],
                                in_=qk_ps, func=AF.Identity,
                                bias=bcol[:, ob:ob + 1],
                            )

                for lt in range(LT):
                    v_ps = ps_qkv.tile([128, 512], F32, name="v_ps",
                                       tag=f"v{lt % 3}", bufs=1)
                    for kp in range(2):
                        nc.tensor.matmul(
                            v_ps,
                            xn[kp][:, :, lt * 128:(lt + 1) * 128],
                            w_sb[("wv", kp)],
                            start=(kp == 0), stop=False,
                            perf_mode=DR,
                        )
                    nc.tensor.matmul(
                        v_ps, ones_row[:, 0:128], brow["bv"],
                        start=False, stop=True,
                    )
                    nc.scalar.activation(
                        out=vhat[lt // 2].rearrange(
                            "p s (h c) -> p s h c", c=VH)[:, lt % 2, :, 0:CH],
                        in_=v_ps.rearrange("p (h c) -> p h c", c=CH),
                        func=AF.Identity,
                    )

            # residual carries the proj bias: x += bp (emitted here so the
            # DVE work lands in the qkv phase, not the startup window)
            for cb in range(CB):
                nc.vector.tensor_scalar(
                    out=x_sb[cb], in0=x_sb[cb],
                    scalar1=bp_col[:, cb:cb + 1], scalar2=None,
                    op0=ALU.add)

            # ---------- attention ----------
            # a in DR slab layout for proj: [kp][128, 2, L]
            a_pair = [per.tile([128, 2, L], F8, name=f"ap{i}") for i in range(2)]
            with tc.tile_pool(name="ps_att", bufs=1, space="PSUM") as ps_att:
                sci = [0]
                for hp in range(N_HEADS // 2):
                    aun_sb = {}
                    for sub in range(2):
                        aun_sb[sub] = tmp.tile([VH, L], F32, name=f"aunsb{sub}",
                                               tag=f"aunsb{sub}", bufs=2)
                    for hf in range(2):
                        aun_ps = {}
                        for sub in range(2):
                            aun_ps[sub] = ps_att.tile([VH, 512], F32,
                                                      name=f"aun{sub}",
                                                      tag=f"aun{sub}", bufs=2)
                        for sp in range(LT // 2):
                            ex8 = {}
                            for sub in range(2):
                                ex8[sub] = tmp.tile([128, 2, 512], F8,
                                                    name="ex8",
                                                    tag=f"ex{sub}", bufs=2)
                            for stp in range(2):
                                st = 2 * sp + stp
                                for sub in range(2):
                                    pl = 64 * sub
                                    sc = ps_att.tile(
                                        [128, 512], F32, name="sc",
                                        tag=f"sc{sci[0] % 4}", bufs=1)
                                    sci[0] += 1
                                    nc.tensor.matmul(
                                        sc,
                                        k_sb[hp][pl:pl + 64,
                                                 st * 128:(st + 1) * 128],
                                        q_sb[hp][pl:pl + 64,
                                                 hf * 512:(hf + 1) * 512],
                                        start=True, stop=True,
                                        tile_position=(pl, 0),
                                    )
                                    if exp_engine() == "act":
                                        nc.scalar.activation(
                                            out=ex8[sub][:, stp, :], in_=sc,
                                            func=AF.Exp, scale=SC_SCL,
                                            bias=expb_col,
                                        )
                                    else:
                                        nc.vector.tensor_scalar(
                                            out=ex8[sub][:, stp, :].bitcast(I8),
                                            in0=sc, scalar1=KS_DVE,
                                            scalar2=BS_DVE,
                                            op0=ALU.mult, op1=ALU.add,
                                        )
                            for sub in range(2):
                                h = 2 * hp + sub
                                nc.tensor.matmul(
                                    aun_ps[sub][:, :],
                                    vhat[sp].rearrange(
                                        "p s (h c) -> p s h c", c=VH)[:, :, h, :],
                                    ex8[sub],
                                    start=(sp == 0), stop=(sp == LT // 2 - 1),
                                    perf_mode=DR,
                                )
                        for sub in range(2):
                            nc.scalar.activation(
                                out=aun_sb[sub][:, hf * 512:(hf + 1) * 512],
                                in_=aun_ps[sub], func=AF.Identity,
                            )

                    # denominator rows -> DRAM -> partition-broadcast back
                    # (engines cannot cross partitions, so the raw denominator
                    # is bounced through DRAM and the reciprocal happens on the
                    # broadcast [64, L] tile at base partition 0)
                    for sub in range(2):
                        h = 2 * hp + sub
                        nc.sync.dma_start(out=recip_d.ap()[h:h + 1, :],
                                            in_=aun_sb[sub][CH:CH + 1, :])
                    for sub in range(2):
                        h = 2 * hp + sub
                        kp, slab = h // 4, (h // 2) % 2
                        den_b = tmp.tile([CH, L], F32, name="den_b",
                                         tag=f"den{sub}", bufs=2)
                        bsrc = bass.AP(
                            tensor=recip_d.ap().tensor, offset=h * L,
                            ap=[[0, CH], [1, L]],
                        )
                        nc.sync.dma_start(out=den_b, in_=bsrc)
                        rec_b = tmp.tile([CH, L], F32, name="rec_b",
                                         tag=f"recb{sub}", bufs=2)
                        nc.vector.reciprocal_approx_fast(out=rec_b, in_=den_b)
                        if h % 2 == 0:
                            nc.vector.scalar_tensor_tensor(
                                out=a_pair[kp][0:CH, slab, :],
                                in0=aun_sb[sub][0:CH, :], scalar=SAB,
                                in1=rec_b, op0=ALU.mult, op1=ALU.mult,
                            )
                        else:
                            ahead = tmp.tile([CH, L], F8, name="ahead",
                                             tag="ahead", bufs=2)
                            nc.vector.scalar_tensor_tensor(
                                out=ahead, in0=aun_sb[sub][0:CH, :],
                                scalar=SAB, in1=rec_b,
                                op0=ALU.mult, op1=ALU.mult,
                            )
                            nc.sync.dma_start(
                                out=a_pair[kp][CH:128, slab, :], in_=ahead)

                # ---------- proj + residual (two waves; kp0 prefires
                # against a_pair[0] while the last pair's divisions run) ----
                for wave in range(2):
                    o_tiles = {}
                    for g in range(4):
                        ob, hf = (2 * wave + g // 2), g % 2
                        o_tiles[g] = ps_att.tile([128, 512], F32, name="o_ps",
                                                 tag=f"sc{g}", bufs=1)
                        nc.tensor.matmul(
                            o_tiles[g],
                            w_sb[("wp", 0)][:, :, ob * 128:(ob + 1) * 128],
                            a_pair[0][:, :, hf * 512:(hf + 1) * 512],
                            start=True, stop=False,
                            perf_mode=DR,
                        )
                    for g in range(4):
                        ob, hf = (2 * wave + g // 2), g % 2
                        nc.tensor.matmul(
                            o_tiles[g],
                            w_sb[("wp", 1)][:, :, ob * 128:(ob + 1) * 128],
                            a_pair[1][:, :, hf * 512:(hf + 1) * 512],
                            start=False, stop=True,
                            perf_mode=DR,
                        )
                        res = tmp.tile([128, 512], F32, name="res",
                                       tag="res", bufs=3)
                        nc.vector.scalar_tensor_tensor(
                            out=res, in0=o_tiles[g], scalar=1.0 / SPT,
                            in1=x_sb[ob][:, hf * 512:(hf + 1) * 512],
                            op0=ALU.mult, op1=ALU.add,
                        )
                        nc.sync.dma_start(
                            out=out_d.ap()[ob * 128:(ob + 1) * 128,
                                           hf * 512:(hf + 1) * 512],
                            in_=res,
                        )

    nc.compile()
    return nc


def _f8(x):
    return np.ascontiguousarray(x.astype(ml_dtypes.float8_e4m3fn))


def make_in_maps(x, gn_scale, gn_bias, qkv_w, qkv_b, proj_w, proj_b):
    xf = np.ascontiguousarray(np.asarray(x, dtype=np.float32).reshape(B, C, L))
    qkv_w = np.asarray(qkv_w, dtype=np.float32)
    qkv_b = np.asarray(qkv_b, dtype=np.float32)
    proj_w = np.asarray(proj_w, dtype=np.float32)
    proj_b = np.asarray(proj_b, dtype=np.float32)

    def dr_tiles(wT, scl):
        # wT: [c_in, c_out] -> per kp [128, 2*C] fp8 with slab j = blk 2kp+j
        out = {}
        for kp in range(2):
            t = np.stack([wT[256 * kp + 128 * j:256 * kp + 128 * (j + 1)]
                          for j in range(2)], axis=1) * scl
            out[kp] = _f8(t.reshape(128, 2 * C))
        return out

    wq = dr_tiles(qkv_w[0:C].T, SATT * SQ)
    wk = dr_tiles(qkv_w[C:2 * C].T, SATT * SK)
    wv = dr_tiles(qkv_w[2 * C:3 * C].T, SV)
    wp = dr_tiles(proj_w.T, SPW)

    common = {
        "bq": np.ascontiguousarray((qkv_b[0:C] * SATT * SQ).reshape(1, C)),
        "bk": np.ascontiguousarray((qkv_b[C:2 * C] * SATT * SK).reshape(1, C)),
        "bv": _f8((qkv_b[2 * C:3 * C] * SV).reshape(1, C)),
        "bp": np.ascontiguousarray(proj_b.reshape(1, C)),
        "gamma": np.ascontiguousarray(
            np.asarray(gn_scale, dtype=np.float32).reshape(CB, 128)),
        "beta": np.ascontiguousarray(
            np.asarray(gn_bias, dtype=np.float32).reshape(CB, 128)),
    }
    for nm, w in (("wq", wq), ("wk", wk), ("wv", wv), ("wp", wp)):
        for kp in range(2):
            common[f"{nm}{kp}"] = w[kp]
    return [{"x": np.ascontiguousarray(xf[b]), **common} for b in range(B)]


def run(inputs, trace=False, trace_kwargs=None):
    nc = build_program()
    in_maps = make_in_maps(**inputs)
    res = run_bass_kernel_spmd(
        nc, in_maps, list(range(B)), trace=trace, **(trace_kwargs or {})
    )
    out = np.stack([res.results[b]["out"] for b in range(B)], axis=0)
    return out.reshape(B, C, H, W), res


def kernel(**inputs):
    out, _ = run(inputs)
    return out
